# revision 1
# baseline (speedup 1.0000x reference)
"""Causal self-attention with reference-feature cross keys, on 8 TRN2 cores.

Sharding: tensor-parallel over heads. Core c owns global heads (2c, 2c+1),
i.e. columns [128c:128c+128) of Wq/Wk/Wv/Wrk/Wrv and rows [128c:128c+128)
of Wp. Each core returns a partial y (bf16); the host sums the 8 partials
and adds bp (the "all-reduce").

All-bf16 datapath (tol is 2e-2; bf16 end-to-end lands ~1e-2):
  - x/ref fed pre-transposed and chunk-contiguous ([b*n, 128 partitions,
    8 co-chunks, 512 tokens] bf16) so each 512-token chunk loads with one
    128x8KB-contiguous DMA; weights bf16. PSUM accumulation f32.
  - bk/brk dropped: the score term q.bk is constant along keys -> softmax
    invariant (cancels exactly in num/denom).
  - Diagonal causal blocks are query-range restricted: for key block r of
    a 512-token chunk only queries t >= 128r participate; only the
    128x128 triangle straddling the diagonal needs an actual mask
    (multiplicative bf16 triangle on DVE). Fully-masked regions are never
    computed; PV covers them via the (full-range) ref block that opens
    the PSUM accumulation group.
  - exp without max-subtraction (|S/8| < ~3).

Engine schedule: PE is the bottleneck (~75 us/batch), so all non-attention
PE work is kept in filler queues drained between attention blocks:
  - indep queue: next batch's projections (q/k/v/ref) — no deps on the
    current batch's attention; drained preferentially right after each
    chunk's warmup to cover the po-bank + normalize-chain stall.
  - yq queue: output projection (OT_blk.T @ Wp rows) of finished chunks.
The per-block S->exp->PV pipeline (DEPTH=2) otherwise keeps PE ahead of
ACT's exp stream (ACT is #2 at ~53 us/batch).
"""
import sys

sys.path.insert(0, "/opt/trn_rl_repo")

import numpy as np

B, T, C = 4, 2048, 1024
TR = 512
D = 64
DC = 128          # per-core slice of C (2 heads x 64)
H_PER = 2
NCH = T // 512    # 512-token chunks per batch
NCORES = 8

_CACHE = {}


COPY_ENGINE = "any"   # "any" | "vector"
MASK_ENGINE = "vector"  # "vector" | "gpsimd"


def _build_program(repeat=1, ablate="none"):
    import concourse.bacc as bacc
    import concourse.mybir as mybir
    import concourse.tile as tile
    from concourse.masks import make_identity

    F32 = mybir.dt.float32
    BF16 = mybir.dt.bfloat16
    AF = mybir.ActivationFunctionType
    OP = mybir.AluOpType

    nc = bacc.Bacc("TRN2", target_bir_lowering=False, debug=False,
                   num_devices=NCORES)
    cp = nc.any if COPY_ENGINE == "any" else nc.vector
    mk = nc.vector if MASK_ENGINE == "vector" else nc.gpsimd

    xt_d = nc.dram_tensor("xt", [B * NCH, 128, 8, 512], BF16,
                      kind="ExternalInput").ap()
    rt_d = nc.dram_tensor("rt", [B, 128, 8, 512], BF16,
                          kind="ExternalInput").ap()
    w_d = {}
    for nm in ("wq", "wk", "wv", "wrk", "wrv"):
        w_d[nm] = nc.dram_tensor(nm, [C, DC], BF16, kind="ExternalInput").ap()
    wp_d = nc.dram_tensor("wp", [DC, C], BF16, kind="ExternalInput").ap()
    b_d = {}
    for nm in ("bq", "bv", "brv"):
        b_d[nm] = nc.dram_tensor(nm, [DC], F32, kind="ExternalInput").ap()
    tri_d = nc.dram_tensor("tri", [128, 128], BF16, kind="ExternalInput").ap()
    out_d = nc.dram_tensor("out", [B, T, C], BF16, kind="ExternalOutput").ap()


    with tile.TileContext(nc) as tc:
        with (
            tc.tile_pool(name="const", bufs=1) as constp,
            tc.tile_pool(name="work", bufs=2) as work,
            tc.tile_pool(name="psum", bufs=1, space="PSUM") as psp,
        ):
            ident = constp.tile([128, 128], BF16)
            make_identity(nc, ident[:])
            tri = constp.tile([128, 128], BF16)
            nc.sync.dma_start(tri[:], tri_d)
            ones_col = constp.tile([128, 16], BF16)
            nc.any.memset(ones_col[:], 1.0)

            w_sb = {}
            for nm in ("wq", "wk", "wv", "wrk", "wrv"):
                w = constp.tile([128, 8, DC], BF16, name=f"{nm}_sb")
                nc.sync.dma_start(w[:], w_d[nm].rearrange("(co p) m -> p co m", p=128))
                w_sb[nm] = w
            wp_r = constp.tile([DC, C], BF16)
            nc.sync.dma_start(wp_r[:], wp_d)

            b_sb = {}
            for nm in ("bq", "bv", "brv"):
                bias = constp.tile([DC, 1], F32, name=f"{nm}_sb")
                nc.sync.dma_start(bias[:], b_d[nm].unsqueeze(1))
                b_sb[nm] = bias

            # PE filler queues, drained between attention blocks.
            indep = []   # next batch's projections: no attention deps
            yq = []      # output projections: depend on finished OT chunks

            def drain(prefer_indep=False):
                if prefer_indep and indep:
                    indep.pop(0)()
                elif yq:
                    yq.pop(0)()
                elif indep:
                    indep.pop(0)()

            def project_half(xT, wname, bname, dst, half, cell):
                """Half of dst[128, 512] (bf16) = W.T @ xT (+ bias): co-chunks
                half*4..half*4+3; the second half adds bias and copies out."""
                if half == 0:
                    cell.append(psp.tile([128, 512], F32, tag="pp", bufs=2,
                                         name="pp"))
                pp = cell[0]
                for co in range(4 * half, 4 * half + 4):
                    nc.tensor.matmul(pp[:], w_sb[wname][:, co, :],
                                     xT[:, co, :], start=(co == 0), stop=(co == 7))
                if half == 1:
                    if bname is None:
                        cp.tensor_copy(dst, pp[:])
                    else:
                        cp.tensor_scalar_add(dst, pp[:], b_sb[bname][:])

            def project(xT, wname, bname, dst):
                """dst[128, 512] (bf16) = W.T @ xT (+ bias)."""
                if ablate == "noproj":
                    cp.tensor_copy(dst, xT[:, 0, :])
                    return
                cell = []
                project_half(xT, wname, bname, dst, 0, cell)
                project_half(xT, wname, bname, dst, 1, cell)

            def v_natural(vT, dst_vsb, j0):
                """Transpose vT [128, 512] into v_sb blocks j0..j0+3 (+ones cols)."""
                pt = psp.tile([128, 512], BF16, tag="pp", bufs=2)
                for a in range(4):
                    nc.tensor.transpose(
                        pt[:, 128 * a:128 * (a + 1)],
                        vT[:, 128 * a:128 * (a + 1)], ident[:])
                ptv = pt[:].rearrange("p (a m) -> p a m", a=4)
                cp.tensor_copy(dst_vsb[:, j0:j0 + 4, 0:64], ptv[:, :, 0:64])
                cp.tensor_copy(dst_vsb[:, j0:j0 + 4, 66:130], ptv[:, :, 64:128])

            def stage_proj(b, st=None):
                """Allocate batch b's tiles (or reuse `st`), issue its input
                DMAs, and return (tiles, closures) where the closures emit
                the projection compute when drained."""
                if st is None:
                    st = {
                        "qT": work.tile([128, NCH, 512], BF16, tag="qT",
                                        name="qT", bufs=3),
                        "kT": work.tile([128, NCH, 512], BF16, tag="kT",
                                        name="kT", bufs=3),
                        "v_sb": work.tile([128, 4 * NCH, 132], BF16, tag="vsb",
                                          name="v_sb", bufs=3),
                        "rkT": work.tile([128, 512], BF16, tag="rkT",
                                         name="rkT", bufs=3),
                        "rv_sb": work.tile([128, 4, 132], BF16, tag="rvsb",
                                           name="rv_sb", bufs=3),
                        "OT": work.tile([128, NCH, 512], BF16, tag="OT",
                                        name="OT", bufs=3),
                    }
                nc.vector.tensor_copy(st["v_sb"][:, :, 64:65], ones_col[:, :, None])
                nc.vector.tensor_copy(st["v_sb"][:, :, 130:131], ones_col[:, :, None])
                nc.vector.tensor_copy(st["rv_sb"][:, :, 64:65], ones_col[:, 0:4, None])
                nc.vector.tensor_copy(st["rv_sb"][:, :, 130:131], ones_col[:, 0:4, None])
                xTs = []
                for n in range(NCH):
                    xT = work.tile([128, 8, 512], BF16, tag="xT", bufs=5)
                    nc.sync.dma_start(xT[:], xt_d[b * NCH + n])
                    xTs.append(xT)
                refT = work.tile([128, 8, 512], BF16, tag="xT", bufs=5)
                nc.sync.dma_start(refT[:], rt_d[b])

                cl = []
                if ablate == "noproj":
                    for n in range(NCH):
                        xT = xTs[n]
                        cl.append(lambda xT=xT, n=n:
                                  project(xT, "wq", "bq", st["qT"][:, n, :]))
                        cl.append(lambda xT=xT, n=n:
                                  project(xT, "wk", None, st["kT"][:, n, :]))

                        def vwork(xT=xT, n=n):
                            vT = work.tile([128, 512], BF16, tag="vT")
                            project(xT, "wv", "bv", vT[:])
                            v_natural(vT, st["v_sb"], 4 * n)
                        cl.append(vwork)
                    cl.append(lambda: project(refT, "wrk", None, st["rkT"][:]))

                    def rvwork():
                        vT = work.tile([128, 512], BF16, tag="vT")
                        project(refT, "wrv", "brv", vT[:])
                        v_natural(vT, st["rv_sb"], 0)
                    cl.append(rvwork)
                    return st, cl

                def halves(xT, wname, bname, dst):
                    cell = []
                    return [
                        lambda: project_half(xT, wname, bname, dst, 0, cell),
                        lambda: project_half(xT, wname, bname, dst, 1, cell),
                    ]

                vTs = {}
                for n in range(NCH):
                    xT = xTs[n]
                    cl += halves(xT, "wq", "bq", st["qT"][:, n, :])
                    cl += halves(xT, "wk", None, st["kT"][:, n, :])
                    vT = work.tile([128, 512], BF16, tag="vT", name="vT")
                    cl += halves(xT, "wv", "bv", vT[:])
                    cl.append(lambda vT=vT, n=n:
                              v_natural(vT, st["v_sb"], 4 * n))
                cl += halves(refT, "wrk", None, st["rkT"][:])
                rvT = work.tile([128, 512], BF16, tag="vT", name="rvT")
                cl += halves(refT, "wrv", "brv", rvT[:])
                cl.append(lambda: v_natural(rvT, st["rv_sb"], 0))
                return st, cl

            def yproj_closures(b, c, OT):
                """Output projection for chunk (b, c): 4 token blocks x 2
                column halves; one contiguous 2KB-row DMA per token block."""
                def emit(a, half, cell, OT=OT):
                    stat = OT[:, c, 128 * a:128 * (a + 1)]
                    py = psp.tile([128, 512], F32, tag="pp", bufs=2)
                    nc.tensor.matmul(py[:], stat,
                                     wp_r[:, 512 * half:512 * (half + 1)],
                                     start=True, stop=True)
                    if half == 0:
                        cell.append(work.tile([128, 1024], BF16, tag="y",
                                              bufs=3, name="y_sb"))
                    y_sb = cell[0]
                    cp.tensor_copy(y_sb[:, 512 * half:512 * (half + 1)], py[:])
                    if half == 1:
                        t0 = 512 * c + 128 * a
                        nc.sync.dma_start(out_d[b, t0:t0 + 128, :], y_sb[:])
                cls = []
                for a in range(4):
                    cell = []
                    cls.append(lambda a=a, cell=cell: emit(a, 0, cell))
                    cls.append(lambda a=a, cell=cell: emit(a, 1, cell))
                return cls

            DEPTH = 3

            def attention_batch(b, st):
                qT, kT, v_sb = st["qT"], st["kT"], st["v_sb"]
                rkT, rv_sb, OT = st["rkT"], st["rv_sb"], st["OT"]
                if ablate == "noattn":
                    for c in range(NCH):
                        nc.vector.tensor_copy(OT[:, c, :], qT[:, c, :])
                        yq.extend(yproj_closures(b, c, OT))
                    return
                for c in range(NCH):
                    po_t = psp.tile([128, 2, 512], F32, tag="po", bufs=1,
                                    name="po_t")
                    po = [po_t[:, 0, :], po_t[:, 1, :]]
                    # ref blocks (full range; ref0 opens the PSUM group),
                    # self full blocks, diag r=3..1 (query-restricted),
                    # diag r=0 last (full range, carries the stop flag).
                    blocks = [("ref", jr, 0) for jr in range(4)]
                    blocks += [("self", j, 0) for j in range(4 * c)]
                    blocks += [("diag", 4 * c + r, 128 * r) for r in (3, 2, 1, 0)]
                    nb = len(blocks)
                    Es = {}

                    def s_stage(bi, c=c, blocks=blocks, Es=Es):
                        kind, j, qr = blocks[bi]
                        ps = psp.tile([128, 2, 512], F32, tag="s", bufs=2)
                        for h in (() if ablate == "nos" else range(H_PER)):
                            if kind == "ref":
                                stat = rkT[64 * h:64 * (h + 1),
                                           128 * j:128 * (j + 1)]
                            else:
                                stat = kT[64 * h:64 * (h + 1), j // 4,
                                          128 * (j % 4):128 * (j % 4 + 1)]
                            nc.tensor.matmul(ps[:, h, qr:512], stat,
                                             qT[64 * h:64 * (h + 1), c, qr:512],
                                             start=True, stop=True)
                        E = work.tile([128, 2, 512], BF16, tag="E",
                                      bufs=DEPTH + 6)
                        if ablate == "noexp":
                            nc.vector.tensor_copy(E[:, :, qr:512], ps[:, :, qr:512])
                        else:
                            nc.scalar.activation(E[:, :, qr:512], ps[:, :, qr:512],
                                                 AF.Exp, scale=0.125)
                        if kind == "diag":
                            mk.tensor_tensor(
                                E[:, :, qr:qr + 128], E[:, :, qr:qr + 128],
                                tri[:, None, :].to_broadcast((128, 2, 128)),
                                OP.mult)
                        Es[bi] = E

                    def pv_stage(bi, blocks=blocks, Es=Es, po=po, nb=nb,
                                 v_sb=v_sb, rv_sb=rv_sb):
                        kind, j, qr = blocks[bi]
                        E = Es.pop(bi)
                        if ablate == "nopv":
                            if bi == 0:
                                for h in range(H_PER):
                                    nc.tensor.matmul(po[h][0:65, :],
                                                     v_sb[:, 0, 66 * h:66 * h + 65],
                                                     E[:, h, :],
                                                     start=True, stop=True)
                            return
                        for h in range(H_PER):
                            vstat = (rv_sb[:, j, 66 * h:66 * h + 65]
                                     if kind == "ref"
                                     else v_sb[:, j, 66 * h:66 * h + 65])
                            nc.tensor.matmul(po[h][0:65, qr:512], vstat,
                                             E[:, h, qr:512],
                                             start=(bi == 0), stop=(bi == nb - 1))

                    for bi in range(min(DEPTH, nb)):
                        s_stage(bi)
                    # cover the previous chunk's normalize/po-release stall
                    # with attention-independent work when available
                    drain(prefer_indep=True)
                    drain(prefer_indep=True)
                    for bi in range(nb):
                        if bi + DEPTH < nb:
                            s_stage(bi + DEPTH)
                        pv_stage(bi)
                        if bi < nb - 1:
                            drain()
                            if b == B - 1:
                                drain()
                    # free the po banks with a single PSUM->SBUF copy, then
                    # normalize off the critical path from the copy
                    poc = work.tile([128, 2, 512], F32, tag="poc", bufs=2)
                    nc.vector.tensor_copy(poc[0:65, :, :], po_t[0:65, :, :])
                    recs = []
                    for h in range(H_PER):
                        rec = work.tile([1, 512], F32, tag="rec", bufs=2)
                        with nc.allow_low_precision(reason="softmax denom recip"):
                            nc.vector.reciprocal(rec[:], poc[64:65, h, :])
                        recs.append(rec)
                    bcs = []
                    for h in range(H_PER):
                        bc_sb = work.tile([64, 512], F32, tag="bc", bufs=2)
                        nc.gpsimd.partition_broadcast(bc_sb[:], recs[h][:])
                        bcs.append(bc_sb)
                    for h in range(H_PER):
                        nc.vector.tensor_tensor(OT[64 * h:64 * (h + 1), c, :],
                                                poc[0:64, h, :], bcs[h][:], OP.mult)
                    yq.extend(yproj_closures(b, c, OT))

            import contextlib
            rep_ctx = (tc.For_i(0, repeat, 1,
                       hint_engines=(mybir.EngineType.PE,
                                     mybir.EngineType.Activation,
                                     mybir.EngineType.DVE,
                                     mybir.EngineType.Pool,
                                     mybir.EngineType.SP))
               if repeat > 1 else contextlib.nullcontext())
            # prologue: batch 0's projections run inline, once
            st0, cl = stage_proj(0)
            for fn in cl:
                fn()
            with rep_ctx:
                st = st0
                for b in range(B):
                    st_next = None
                    if b + 1 < B:
                        st_next, cl_next = stage_proj(b + 1)
                        indep.extend(cl_next)
                    elif repeat > 1:
                        # software-pipeline the repeat loop: re-stage batch
                        # 0's projections (next iteration) into b3's attention
                        _, cl_next = stage_proj(0, st=st0)
                        indep.extend(cl_next)
                    attention_batch(b, st)
                    # correctness: batch b+1's attention reads tiles written
                    # by these closures, so they must be emitted before it
                    while indep:
                        indep.pop(0)()
                    st = st_next
                while yq:
                    yq.pop(0)()

    nc.compile()
    return nc


def _get_program(repeat=1, ablate="none"):
    key = ("nc", repeat, ablate)
    if key not in _CACHE:
        _CACHE[key] = _build_program(repeat, ablate)
    return _CACHE[key]


def _make_tri():
    s = np.arange(128)[:, None]
    t = np.arange(128)[None, :]
    return (t >= s).astype(np.float32)


def make_in_maps(x, ref_feat, Wq, bq, Wk, bk, Wv, bv, Wrk, brk, Wrv, brv, Wp, bp):
    import ml_dtypes
    bf16 = ml_dtypes.bfloat16

    x = np.asarray(x, dtype=np.float32)
    ref_feat = np.asarray(ref_feat, dtype=np.float32)
    # [b, n, p, co, t]: each 512-token chunk is partition-contiguous
    xt = np.ascontiguousarray(
        x.reshape(B * NCH, 512, 8, 128).transpose(0, 3, 2, 1)).astype(bf16)
    rt = np.ascontiguousarray(
        ref_feat.reshape(B, 512, 8, 128).transpose(0, 3, 2, 1)).astype(bf16)
    tri = _make_tri().astype(bf16)

    in_maps = []
    for c in range(NCORES):
        sl = slice(DC * c, DC * (c + 1))
        in_maps.append({
            "xt": xt, "rt": rt, "tri": tri,
            "wq": np.ascontiguousarray(np.asarray(Wq)[:, sl]).astype(bf16),
            "wk": np.ascontiguousarray(np.asarray(Wk)[:, sl]).astype(bf16),
            "wv": np.ascontiguousarray(np.asarray(Wv)[:, sl]).astype(bf16),
            "wrk": np.ascontiguousarray(np.asarray(Wrk)[:, sl]).astype(bf16),
            "wrv": np.ascontiguousarray(np.asarray(Wrv)[:, sl]).astype(bf16),
            "wp": np.ascontiguousarray(np.asarray(Wp)[sl, :]).astype(bf16),
            "bq": np.ascontiguousarray(np.asarray(bq)[sl]).astype(np.float32),
            "bv": np.ascontiguousarray(np.asarray(bv)[sl]).astype(np.float32),
            "brv": np.ascontiguousarray(np.asarray(brv)[sl]).astype(np.float32),
        })
    return in_maps


def kernel(x, ref_feat, Wq, bq, Wk, bk, Wv, bv, Wrk, brk, Wrv, brv, Wp, bp):
    from concourse.bass_utils import run_bass_kernel_spmd

    nc = _get_program()
    in_maps = make_in_maps(x, ref_feat, Wq, bq, Wk, bk, Wv, bv,
                           Wrk, brk, Wrv, brv, Wp, bp)
    res = run_bass_kernel_spmd(nc, in_maps, list(range(NCORES))).results
    y = res[0]["out"].astype(np.float64)
    for c in range(1, NCORES):
        y += res[c]["out"].astype(np.float64)
    y += np.asarray(bp, dtype=np.float64)
    return y.astype(np.float32)



# revision 9
# speedup vs baseline: 1.0146x; 1.0146x over previous
"""Causal self-attention with reference-feature cross keys, on 8 TRN2 cores.

Sharding: tensor-parallel over heads. Core c owns global heads (2c, 2c+1),
i.e. columns [128c:128c+128) of Wq/Wk/Wv/Wrk/Wrv and rows [128c:128c+128)
of Wp. Each core returns a partial y (bf16); the host sums the 8 partials
and adds bp (the "all-reduce").

All-bf16 datapath (tol is 2e-2; bf16 end-to-end lands ~1e-2):
  - x/ref fed pre-transposed and chunk-contiguous ([b*n, 128 partitions,
    8 co-chunks, 512 tokens] bf16) so each 512-token chunk loads with one
    128x8KB-contiguous DMA; weights bf16. PSUM accumulation f32.
  - bk/brk dropped: the score term q.bk is constant along keys -> softmax
    invariant (cancels exactly in num/denom).
  - Diagonal causal blocks are query-range restricted: for key block r of
    a 512-token chunk only queries t >= 128r participate; only the
    128x128 triangle straddling the diagonal needs an actual mask
    (multiplicative bf16 triangle on DVE). Fully-masked regions are never
    computed; PV covers them via the (full-range) ref block that opens
    the PSUM accumulation group.
  - exp without max-subtraction (|S/8| < ~3).

Engine schedule: PE is the bottleneck (~75 us/batch), so all non-attention
PE work is kept in filler queues drained between attention blocks:
  - indep queue: next batch's projections (q/k/v/ref) — no deps on the
    current batch's attention; drained preferentially right after each
    chunk's warmup to cover the po-bank + normalize-chain stall.
  - yq queue: output projection (OT_blk.T @ Wp rows) of finished chunks.
The per-block S->exp->PV pipeline (DEPTH=2) otherwise keeps PE ahead of
ACT's exp stream (ACT is #2 at ~53 us/batch).
"""
import sys

sys.path.insert(0, "/opt/trn_rl_repo")

import numpy as np

B, T, C = 4, 2048, 1024
TR = 512
D = 64
DC = 128          # per-core slice of C (2 heads x 64)
H_PER = 2
NCH = T // 512    # 512-token chunks per batch
NCORES = 8

_CACHE = {}


COPY_ENGINE = "any"   # "any" | "vector"
MASK_ENGINE = "vector"  # "vector" | "gpsimd"


def _build_program(repeat=1, ablate="none"):
    import concourse.bacc as bacc
    import concourse.mybir as mybir
    import concourse.tile as tile
    from concourse.masks import make_identity

    F32 = mybir.dt.float32
    BF16 = mybir.dt.bfloat16
    AF = mybir.ActivationFunctionType
    OP = mybir.AluOpType

    nc = bacc.Bacc("TRN2", target_bir_lowering=False, debug=False,
                   num_devices=NCORES)
    cp = nc.any if COPY_ENGINE == "any" else nc.vector
    mk = nc.vector if MASK_ENGINE == "vector" else nc.gpsimd

    xt_d = nc.dram_tensor("xt", [B * NCH, 128, 8, 512], BF16,
                      kind="ExternalInput").ap()
    rt_d = nc.dram_tensor("rt", [B, 128, 8, 512], BF16,
                          kind="ExternalInput").ap()
    w_d = {}
    for nm in ("wq", "wk", "wv", "wrk", "wrv"):
        w_d[nm] = nc.dram_tensor(nm, [C, DC], BF16, kind="ExternalInput").ap()
    wp_d = nc.dram_tensor("wp", [DC, C], BF16, kind="ExternalInput").ap()
    b_d = {}
    for nm in ("bq", "bv", "brv"):
        b_d[nm] = nc.dram_tensor(nm, [DC], F32, kind="ExternalInput").ap()
    tri_d = nc.dram_tensor("tri", [128, 128], BF16, kind="ExternalInput").ap()
    out_d = nc.dram_tensor("out", [B, T, C], BF16, kind="ExternalOutput").ap()


    with tile.TileContext(nc) as tc:
        with (
            tc.tile_pool(name="const", bufs=1) as constp,
            tc.tile_pool(name="work", bufs=2) as work,
            tc.tile_pool(name="psum", bufs=1, space="PSUM") as psp,
        ):
            ident = constp.tile([128, 128], BF16)
            make_identity(nc, ident[:])
            tri = constp.tile([128, 128], BF16)
            nc.sync.dma_start(tri[:], tri_d)
            ones_col = constp.tile([128, 16], BF16)
            nc.any.memset(ones_col[:], 1.0)

            w_sb = {}
            for nm in ("wq", "wk", "wv", "wrk", "wrv"):
                w = constp.tile([128, 8, DC], BF16, name=f"{nm}_sb")
                nc.sync.dma_start(w[:], w_d[nm].rearrange("(co p) m -> p co m", p=128))
                w_sb[nm] = w
            wp_r = constp.tile([DC, C], BF16)
            nc.sync.dma_start(wp_r[:], wp_d)

            b_sb = {}
            for nm in ("bq", "bv", "brv"):
                bias = constp.tile([DC, 1], F32, name=f"{nm}_sb")
                nc.sync.dma_start(bias[:], b_d[nm].unsqueeze(1))
                b_sb[nm] = bias

            # PE filler queues, drained between attention blocks.
            indep = []   # next batch's projections: no attention deps
            yq = []      # output projections: depend on finished OT chunks

            def drain(prefer_indep=False):
                if prefer_indep and indep:
                    indep.pop(0)()
                elif yq:
                    yq.pop(0)()
                elif indep:
                    indep.pop(0)()

            def project_half(xT, wname, bname, dst, half, cell):
                """Half of dst[128, 512] (bf16) = W.T @ xT (+ bias): co-chunks
                half*4..half*4+3; the second half adds bias and copies out."""
                if half == 0:
                    cell.append(psp.tile([128, 512], F32, tag="pp", bufs=2,
                                         name="pp"))
                pp = cell[0]
                for co in range(4 * half, 4 * half + 4):
                    nc.tensor.matmul(pp[:], w_sb[wname][:, co, :],
                                     xT[:, co, :], start=(co == 0), stop=(co == 7))
                if half == 1:
                    if bname is None:
                        cp.tensor_copy(dst, pp[:])
                    else:
                        cp.tensor_scalar_add(dst, pp[:], b_sb[bname][:])

            def project(xT, wname, bname, dst):
                """dst[128, 512] (bf16) = W.T @ xT (+ bias)."""
                if ablate == "noproj":
                    cp.tensor_copy(dst, xT[:, 0, :])
                    return
                cell = []
                project_half(xT, wname, bname, dst, 0, cell)
                project_half(xT, wname, bname, dst, 1, cell)

            def v_natural(vT, dst_vsb, j0):
                """Transpose vT [128, 512] into v_sb blocks j0..j0+3 (+ones cols)."""
                pt = psp.tile([128, 512], BF16, tag="pp", bufs=2)
                for a in range(4):
                    nc.tensor.transpose(
                        pt[:, 128 * a:128 * (a + 1)],
                        vT[:, 128 * a:128 * (a + 1)], ident[:])
                ptv = pt[:].rearrange("p (a m) -> p a m", a=4)
                cp.tensor_copy(dst_vsb[:, j0:j0 + 4, 0:64], ptv[:, :, 0:64])
                cp.tensor_copy(dst_vsb[:, j0:j0 + 4, 66:130], ptv[:, :, 64:128])

            def stage_proj(b, st=None):
                """Allocate batch b's tiles (or reuse `st`), issue its input
                DMAs, and return (tiles, closures) where the closures emit
                the projection compute when drained."""
                if st is None:
                    st = {
                        "qT": work.tile([128, NCH, 512], BF16, tag="qT",
                                        name="qT", bufs=3),
                        "kT": work.tile([128, NCH, 512], BF16, tag="kT",
                                        name="kT", bufs=3),
                        "v_sb": work.tile([128, 4 * NCH, 132], BF16, tag="vsb",
                                          name="v_sb", bufs=3),
                        "rkT": work.tile([128, 512], BF16, tag="rkT",
                                         name="rkT", bufs=3),
                        "rv_sb": work.tile([128, 4, 132], BF16, tag="rvsb",
                                           name="rv_sb", bufs=3),
                        "OT": work.tile([128, NCH, 512], BF16, tag="OT",
                                        name="OT", bufs=3),
                    }
                nc.vector.tensor_copy(st["v_sb"][:, :, 64:65], ones_col[:, :, None])
                nc.vector.tensor_copy(st["v_sb"][:, :, 130:131], ones_col[:, :, None])
                nc.vector.tensor_copy(st["rv_sb"][:, :, 64:65], ones_col[:, 0:4, None])
                nc.vector.tensor_copy(st["rv_sb"][:, :, 130:131], ones_col[:, 0:4, None])
                xTs = []
                for n in range(NCH):
                    xT = work.tile([128, 8, 512], BF16, tag="xT", bufs=5)
                    nc.sync.dma_start(xT[:], xt_d[b * NCH + n])
                    xTs.append(xT)
                refT = work.tile([128, 8, 512], BF16, tag="xT", bufs=5)
                nc.sync.dma_start(refT[:], rt_d[b])

                cl = []
                if ablate == "noproj":
                    for n in range(NCH):
                        xT = xTs[n]
                        cl.append(lambda xT=xT, n=n:
                                  project(xT, "wq", "bq", st["qT"][:, n, :]))
                        cl.append(lambda xT=xT, n=n:
                                  project(xT, "wk", None, st["kT"][:, n, :]))

                        def vwork(xT=xT, n=n):
                            vT = work.tile([128, 512], BF16, tag="vT")
                            project(xT, "wv", "bv", vT[:])
                            v_natural(vT, st["v_sb"], 4 * n)
                        cl.append(vwork)
                    cl.append(lambda: project(refT, "wrk", None, st["rkT"][:]))

                    def rvwork():
                        vT = work.tile([128, 512], BF16, tag="vT")
                        project(refT, "wrv", "brv", vT[:])
                        v_natural(vT, st["rv_sb"], 0)
                    cl.append(rvwork)
                    return st, cl

                def halves(xT, wname, bname, dst):
                    cell = []
                    return [
                        lambda: project_half(xT, wname, bname, dst, 0, cell),
                        lambda: project_half(xT, wname, bname, dst, 1, cell),
                    ]

                vTs = {}
                for n in range(NCH):
                    xT = xTs[n]
                    cl += halves(xT, "wq", "bq", st["qT"][:, n, :])
                    cl += halves(xT, "wk", None, st["kT"][:, n, :])
                    vT = work.tile([128, 512], BF16, tag="vT", name="vT")
                    cl += halves(xT, "wv", "bv", vT[:])
                    cl.append(lambda vT=vT, n=n:
                              v_natural(vT, st["v_sb"], 4 * n))
                cl += halves(refT, "wrk", None, st["rkT"][:])
                rvT = work.tile([128, 512], BF16, tag="vT", name="rvT")
                cl += halves(refT, "wrv", "brv", rvT[:])
                cl.append(lambda: v_natural(rvT, st["rv_sb"], 0))
                return st, cl

            def yproj_closures(b, c, OT):
                """Output projection for chunk (b, c): 4 token blocks x 2
                column halves, gathered into one per-chunk tile and written
                out with a single 512-row DMA (fewer SP-queue round trips)."""
                cell = []

                def emit(a, half, OT=OT):
                    stat = OT[:, c, 128 * a:128 * (a + 1)]
                    py = psp.tile([128, 512], F32, tag="pp", bufs=2)
                    nc.tensor.matmul(py[:], stat,
                                     wp_r[:, 512 * half:512 * (half + 1)],
                                     start=True, stop=True)
                    if not cell:
                        cell.append(work.tile([128, 4, 1024], BF16, tag="y",
                                              bufs=2, name="y_sb"))
                    y_sb = cell[0]
                    cp.tensor_copy(y_sb[:, a, 512 * half:512 * (half + 1)],
                                   py[:])
                    if a == 3 and half == 1:
                        t0 = 512 * c
                        nc.sync.dma_start(
                            out_d[b, t0:t0 + 512, :].rearrange(
                                "(a p) m -> p a m", p=128),
                            y_sb[:])
                cls = []
                for a in range(4):
                    cls.append(lambda a=a: emit(a, 0))
                    cls.append(lambda a=a: emit(a, 1))
                return cls

            DEPTH = 3

            def attention_batch(b, st):
                qT, kT, v_sb = st["qT"], st["kT"], st["v_sb"]
                rkT, rv_sb, OT = st["rkT"], st["rv_sb"], st["OT"]
                if ablate == "noattn":
                    for c in range(NCH):
                        nc.vector.tensor_copy(OT[:, c, :], qT[:, c, :])
                        yq.extend(yproj_closures(b, c, OT))
                    return
                for c in range(NCH):
                    po_t = psp.tile([128, 2, 512], F32, tag="po", bufs=1,
                                    name="po_t")
                    po = [po_t[:, 0, :], po_t[:, 1, :]]
                    # ref blocks (full range; ref0 opens the PSUM group),
                    # self full blocks, diag r=3..1 (query-restricted),
                    # diag r=0 last (full range, carries the stop flag).
                    blocks = [("ref", jr, 0) for jr in range(4)]
                    blocks += [("self", j, 0) for j in range(4 * c)]
                    blocks += [("diag", 4 * c + r, 128 * r) for r in (3, 2, 1, 0)]
                    nb = len(blocks)
                    Es = {}

                    def s_stage(bi, c=c, blocks=blocks, Es=Es):
                        kind, j, qr = blocks[bi]
                        ps = psp.tile([128, 2, 512], F32, tag="s", bufs=2)
                        for h in (() if ablate == "nos" else range(H_PER)):
                            if kind == "ref":
                                stat = rkT[64 * h:64 * (h + 1),
                                           128 * j:128 * (j + 1)]
                            else:
                                stat = kT[64 * h:64 * (h + 1), j // 4,
                                          128 * (j % 4):128 * (j % 4 + 1)]
                            nc.tensor.matmul(ps[:, h, qr:512], stat,
                                             qT[64 * h:64 * (h + 1), c, qr:512],
                                             start=True, stop=True)
                        E = work.tile([128, 2, 512], BF16, tag="E",
                                      bufs=DEPTH + 6)
                        if ablate == "noexp":
                            nc.vector.tensor_copy(E[:, :, qr:512], ps[:, :, qr:512])
                        else:
                            nc.scalar.activation(E[:, :, qr:512], ps[:, :, qr:512],
                                                 AF.Exp, scale=0.125)
                        if kind == "diag":
                            mk.tensor_tensor(
                                E[:, :, qr:qr + 128], E[:, :, qr:qr + 128],
                                tri[:, None, :].to_broadcast((128, 2, 128)),
                                OP.mult)
                        Es[bi] = E

                    def pv_stage(bi, blocks=blocks, Es=Es, po=po, nb=nb,
                                 v_sb=v_sb, rv_sb=rv_sb):
                        kind, j, qr = blocks[bi]
                        E = Es.pop(bi)
                        if ablate == "nopv":
                            if bi == 0:
                                for h in range(H_PER):
                                    nc.tensor.matmul(po[h][0:65, :],
                                                     v_sb[:, 0, 66 * h:66 * h + 65],
                                                     E[:, h, :],
                                                     start=True, stop=True)
                            return
                        for h in range(H_PER):
                            vstat = (rv_sb[:, j, 66 * h:66 * h + 65]
                                     if kind == "ref"
                                     else v_sb[:, j, 66 * h:66 * h + 65])
                            nc.tensor.matmul(po[h][0:65, qr:512], vstat,
                                             E[:, h, qr:512],
                                             start=(bi == 0), stop=(bi == nb - 1))

                    for bi in range(min(DEPTH, nb)):
                        s_stage(bi)
                    # cover the previous chunk's normalize/po-release stall
                    # with attention-independent work when available
                    drain(prefer_indep=True)
                    drain(prefer_indep=True)
                    for bi in range(nb):
                        pv_stage(bi)
                        if bi + DEPTH < nb:
                            s_stage(bi + DEPTH)
                        if bi < nb - 1:
                            drain()
                            if b == B - 1:
                                drain()
                    # reciprocals (DVE) run straight off the PSUM denom row,
                    # in parallel with the bank-freeing PSUM->SBUF copy on
                    # the otherwise-idle Pool engine
                    recs = []
                    for h in range(H_PER):
                        rec = work.tile([1, 512], F32, tag="rec", bufs=2)
                        with nc.allow_low_precision(reason="softmax denom recip"):
                            nc.vector.reciprocal(rec[:], po_t[64:65, h, :])
                        recs.append(rec)
                    poc = work.tile([128, 2, 512], F32, tag="poc", bufs=2)
                    nc.gpsimd.tensor_copy(poc[0:64, :, :], po_t[0:64, :, :])
                    bcs = []
                    for h in range(H_PER):
                        bc_sb = work.tile([64, 512], F32, tag="bc", bufs=2)
                        nc.gpsimd.partition_broadcast(bc_sb[:], recs[h][:])
                        bcs.append(bc_sb)
                    for h in range(H_PER):
                        nc.vector.tensor_tensor(OT[64 * h:64 * (h + 1), c, :],
                                                poc[0:64, h, :], bcs[h][:], OP.mult)
                    yq.extend(yproj_closures(b, c, OT))

            import contextlib
            rep_ctx = (tc.For_i(0, repeat, 1,
                       hint_engines=(mybir.EngineType.PE,
                                     mybir.EngineType.Activation,
                                     mybir.EngineType.DVE,
                                     mybir.EngineType.Pool,
                                     mybir.EngineType.SP))
               if repeat > 1 else contextlib.nullcontext())
            # prologue: batch 0's projections run inline, once
            st0, cl = stage_proj(0)
            for fn in cl:
                fn()
            with rep_ctx:
                st = st0
                for b in range(B):
                    st_next = None
                    if b + 1 < B:
                        st_next, cl_next = stage_proj(b + 1)
                        indep.extend(cl_next)
                    elif repeat > 1:
                        # software-pipeline the repeat loop: re-stage batch
                        # 0's projections (next iteration) into b3's attention
                        _, cl_next = stage_proj(0, st=st0)
                        indep.extend(cl_next)
                    attention_batch(b, st)
                    # correctness: batch b+1's attention reads tiles written
                    # by these closures, so they must be emitted before it
                    if b < B - 1:
                        while indep:
                            indep.pop(0)()
                    else:
                        # tail: alternate restaged projections (may stall on
                        # their fresh xt DMAs) with output projections so PE
                        # always has runnable work
                        while indep or yq:
                            if yq:
                                yq.pop(0)()
                            if indep:
                                indep.pop(0)()
                    st = st_next
                while yq:
                    yq.pop(0)()

    nc.compile()
    return nc


def _get_program(repeat=1, ablate="none"):
    key = ("nc", repeat, ablate)
    if key not in _CACHE:
        _CACHE[key] = _build_program(repeat, ablate)
    return _CACHE[key]


def _make_tri():
    s = np.arange(128)[:, None]
    t = np.arange(128)[None, :]
    return (t >= s).astype(np.float32)


def make_in_maps(x, ref_feat, Wq, bq, Wk, bk, Wv, bv, Wrk, brk, Wrv, brv, Wp, bp):
    import ml_dtypes
    bf16 = ml_dtypes.bfloat16

    x = np.asarray(x, dtype=np.float32)
    ref_feat = np.asarray(ref_feat, dtype=np.float32)
    # [b, n, p, co, t]: each 512-token chunk is partition-contiguous
    xt = np.ascontiguousarray(
        x.reshape(B * NCH, 512, 8, 128).transpose(0, 3, 2, 1)).astype(bf16)
    rt = np.ascontiguousarray(
        ref_feat.reshape(B, 512, 8, 128).transpose(0, 3, 2, 1)).astype(bf16)
    tri = _make_tri().astype(bf16)

    in_maps = []
    for c in range(NCORES):
        sl = slice(DC * c, DC * (c + 1))
        in_maps.append({
            "xt": xt, "rt": rt, "tri": tri,
            "wq": np.ascontiguousarray(np.asarray(Wq)[:, sl]).astype(bf16),
            "wk": np.ascontiguousarray(np.asarray(Wk)[:, sl]).astype(bf16),
            "wv": np.ascontiguousarray(np.asarray(Wv)[:, sl]).astype(bf16),
            "wrk": np.ascontiguousarray(np.asarray(Wrk)[:, sl]).astype(bf16),
            "wrv": np.ascontiguousarray(np.asarray(Wrv)[:, sl]).astype(bf16),
            "wp": np.ascontiguousarray(np.asarray(Wp)[sl, :]).astype(bf16),
            "bq": np.ascontiguousarray(np.asarray(bq)[sl]).astype(np.float32),
            "bv": np.ascontiguousarray(np.asarray(bv)[sl]).astype(np.float32),
            "brv": np.ascontiguousarray(np.asarray(brv)[sl]).astype(np.float32),
        })
    return in_maps


def kernel(x, ref_feat, Wq, bq, Wk, bk, Wv, bv, Wrk, brk, Wrv, brv, Wp, bp):
    from concourse.bass_utils import run_bass_kernel_spmd

    nc = _get_program()
    in_maps = make_in_maps(x, ref_feat, Wq, bq, Wk, bk, Wv, bv,
                           Wrk, brk, Wrv, brv, Wp, bp)
    res = run_bass_kernel_spmd(nc, in_maps, list(range(NCORES))).results
    y = res[0]["out"].astype(np.float64)
    for c in range(1, NCORES):
        y += res[c]["out"].astype(np.float64)
    y += np.asarray(bp, dtype=np.float64)
    return y.astype(np.float32)

